# revision 1
# baseline (speedup 1.0000x reference)
"""Trainium2 Bass kernel: PhaseMultiHeadModel (complex phase attention + complex FF
+ ComplexNorm + vocab readout), SPMD over 8 NeuronCores with collectives.

Sharding (v2, fully sharded — no replicated compute):
  * Phase A/B (embed+phase+attention): head-parallel. Core c handles heads
    {2c, 2c+1} = emb cols [128c, 128c+128) over all 2048 tokens.
  * AllGather #1 (f32, 2.1MB/rank) assembles the full post-attention state.
  * Phase C (complex FF + ComplexNorm): output-dim parallel. Core c computes
    FF output dims [128c, 128c+128) (re+im) streaming the gathered state from
    DRAM; magnitude stats partial sums + AllReduce (16KB); local normalize.
  * AllGather #2 (bf16, 1MB/rank) assembles the normalized state.
  * Phase D (vocab readout): vocab-parallel as before, weights+state in bf16
    (f32 psum accumulation), 4000 vocab cols per core.

Heavy math in f32/f32r (fp22) except FF inputs implicitly f32r and the final
readout in bf16 (tolerance is 2e-2; bf16 readout contributes ~5e-3).
"""

import math

import numpy as np

P = 128
NCORES = 8
B, S, V, D, H = 2, 1024, 32000, 1024, 16
HD = D // H
SB = B * S
NH = H // NCORES        # heads per core (2)
SBLK = 512              # attention block / moving free dim
KT = 16                 # 128-row blocks in the gathered state (=H)
Vc = V // NCORES        # 4000
VCP = 4096              # padded vocab cols per core
NVT = VCP // P          # 32
EPS = 1.0e-5
RG = [list(range(NCORES))]


def build_nc():
    """Build the single-core Bass program (identical on all cores; per-core
    behavior comes entirely from per-core input tensors)."""
    import concourse.bass as bass  # noqa: F401
    import concourse.mybir as mybir
    import concourse.tile as tile
    from concourse import bacc
    from concourse.masks import make_identity

    f32 = mybir.dt.float32
    f32r = mybir.dt.float32r
    bf16 = mybir.dt.bfloat16
    AF = mybir.ActivationFunctionType

    def r(ap):
        return ap.bitcast(f32r)

    nc = bacc.Bacc(num_devices=NCORES)

    mg = nc.dram_tensor("mg", [SB, P], f32, kind="ExternalInput")
    cphc = nc.dram_tensor("cphc", [SB, P], f32, kind="ExternalInput")
    sphc = nc.dram_tensor("sphc", [SB, P], f32, kind="ExternalInput")
    mkt = nc.dram_tensor("mk", [NH, P, P], f32, kind="ExternalInput")
    mvt = nc.dram_tensor("mv", [NH, P, P], f32, kind="ExternalInput")
    stepm = nc.dram_tensor("stepm", [P, 1280], f32, kind="ExternalInput")
    onesd = nc.dram_tensor("ones", [P, P], f32, kind="ExternalInput")
    ffA = nc.dram_tensor("ffA", [KT, P, P], bf16, kind="ExternalInput")
    ffB = nc.dram_tensor("ffB", [KT, P, P], bf16, kind="ExternalInput")
    w2t = nc.dram_tensor("w2t", [NVT, P, KT * P], bf16, kind="ExternalInput")
    bias2 = nc.dram_tensor("bias2", [P, NVT], f32, kind="ExternalInput")
    stmaskt = nc.dram_tensor("stmask", [KT, 2], f32, kind="ExternalInput")
    outv = nc.dram_tensor("outv", [NVT, P, SB], f32, kind="ExternalOutput")

    # collective bounce buffers (internal DRAM); everything split into token
    # halves (= batches) so half-b compute hides under half-a readout
    agh_in = [
        nc.dram_tensor(f"ag1{hn}_in", [NH, P, S], bf16) for hn in ("a", "b")
    ]
    agh_out = [
        nc.dram_tensor(f"ag1{hn}_out", [H, P, S], bf16, addr_space="Shared")
        for hn in ("a", "b")
    ]
    sth_in = [nc.dram_tensor(f"st{hn}_in", [2, S], f32) for hn in ("a", "b")]
    sth_out = [
        nc.dram_tensor(f"st{hn}_out", [H, S], f32, addr_space="Shared")
        for hn in ("a", "b")
    ]
    ag2h_in = [
        nc.dram_tensor(f"ag2{hn}_in", [NH, P, S], bf16) for hn in ("a", "b")
    ]
    ag2h_out = [
        nc.dram_tensor(f"ag2{hn}_out", [H, P, S], bf16, addr_space="Shared")
        for hn in ("a", "b")
    ]

    ctx_lp = nc.allow_low_precision(reason="fp22/bf16 compute is intentional")
    ctx_lp.__enter__()
    with tile.TileContext(nc) as tc:
        with tc.tile_pool(name="const", bufs=1) as cpool:
            ident = cpool.tile([P, P], f32)
            make_identity(nc, ident[:])
            ones_col = cpool.tile([P, 1], f32r)
            nc.sync.dma_start(ones_col[:], r(onesd[:, 0:1]))
            bias_sb = cpool.tile([P, NVT], f32)
            nc.sync.dma_start(bias_sb[:], bias2[:])
            stmask = cpool.tile([KT, 2], f32r)
            nc.sync.dma_start(stmask[:], r(stmaskt[:, :]))

            # ======== Phases A+B: embed + phase + attention (2 local heads)
            with tc.tile_pool(name="zh", bufs=1) as zpool:
                # local head blocks, feature-major: [sr_h(64); si_h(64)] x tok
                zH = zpool.tile([P, NH, SB], f32r)
                stepm_sb = zpool.tile([P, 1280], f32)
                nc.sync.dma_start(stepm_sb[:], stepm[:])

                with (
                    tc.tile_pool(name="p1", bufs=3) as p1,
                    tc.tile_pool(name="p1ps", bufs=4, space="PSUM") as p1ps,
                ):
                    SLAB = 4
                    for si in range(SB // P // SLAB):
                        u0 = si * SLAB * P
                        gs = p1.tile([P, SLAB, P], f32, tag="g")
                        nc.sync.dma_start(
                            gs[:], mg[u0 : u0 + SLAB * P, :].rearrange(
                                "(a p) d -> p a d", p=P
                            )
                        )
                        cps = p1.tile([P, SLAB, P], f32, tag="cpt")
                        nc.sync.dma_start(
                            cps[:], cphc[u0 : u0 + SLAB * P, :].rearrange(
                                "(a p) d -> p a d", p=P
                            )
                        )
                        sps_ = p1.tile([P, SLAB, P], f32, tag="spt")
                        nc.sync.dma_start(
                            sps_[:], sphc[u0 : u0 + SLAB * P, :].rearrange(
                                "(a p) d -> p a d", p=P
                            )
                        )
                        mags = p1.tile([P, SLAB, P], f32, tag="mag")
                        nc.scalar.activation(mags[:], gs[:], AF.Tanh)
                        for a_ in range(SLAB):
                            t0 = u0 + a_ * P
                            mag = mags[:, a_, :]
                            cpt = cps[:, a_, :]
                            spt = sps_[:, a_, :]
                            zt = p1.tile([P, NH, P], f32, tag="zt")
                            for j in range(NH):
                                hs = slice(j * HD, (j + 1) * HD)
                                nc.vector.tensor_mul(
                                    zt[:, j, 0:HD], mag[:, hs], cpt[:, hs]
                                )
                                nc.vector.tensor_mul(
                                    zt[:, j, HD:P], mag[:, hs], spt[:, hs]
                                )
                            for j in range(NH):
                                ps = p1ps.tile([P, P], f32, tag="tp")
                                nc.tensor.transpose(ps[:], zt[:, j, :], ident[:])
                                if j % 2 == 0:
                                    nc.scalar.copy(
                                        zH[:, j, t0 : t0 + P], ps[:]
                                    )
                                else:
                                    nc.vector.tensor_copy(
                                        zH[:, j, t0 : t0 + P], ps[:]
                                    )

                with (
                    tc.tile_pool(name="hd", bufs=1) as hp,
                    tc.tile_pool(name="rot", bufs=2) as rp_,
                    tc.tile_pool(name="exp", bufs=3) as ep,
                    tc.tile_pool(name="sm", bufs=2) as smp,
                    tc.tile_pool(name="agc", bufs=2) as agp,
                    tc.tile_pool(name="stps", bufs=2, space="PSUM") as stps,
                    tc.tile_pool(name="pvps", bufs=2, space="PSUM") as pvps,
                    tc.tile_pool(name="smps", bufs=1, space="PSUM") as smps,
                    tc.tile_pool(name="rpps", bufs=1, space="PSUM") as rpps,
                    tc.tile_pool(name="cbrps", bufs=1, space="PSUM") as cbr,
                    tc.tile_pool(name="cbtps", bufs=2, space="PSUM") as cbt,
                ):
                    k2hs, v2hs = [], []
                    for j in range(NH):
                        mk_sb = rp_.tile([P, P], f32r, tag="mk")
                        nc.sync.dma_start(mk_sb[:], r(mkt[j, :, :]))
                        mv_sb = rp_.tile([P, P], f32r, tag="mv")
                        nc.sync.dma_start(mv_sb[:], r(mvt[j, :, :]))
                        k2h = hp.tile([P, SB], f32r, tag=f"k2h{j}")
                        v2fm = hp.tile([P, SB], f32, tag=f"v2fm{j}")
                        for tg in range(SB // SBLK):
                            sl = slice(tg * SBLK, (tg + 1) * SBLK)
                            kps = cbr.tile([P, SBLK], f32, tag="cb")
                            nc.tensor.matmul(
                                kps[:], lhsT=r(mk_sb[:]), rhs=zH[:, j, sl],
                                start=True, stop=True,
                            )
                            nc.scalar.copy(k2h[:, sl], kps[:])
                            vps = cbr.tile([P, SBLK], f32, tag="cb")
                            nc.tensor.matmul(
                                vps[:], lhsT=r(mv_sb[:]), rhs=zH[:, j, sl],
                                start=True, stop=True,
                            )
                            nc.vector.tensor_copy(v2fm[:, sl], vps[:])
                        v2h = hp.tile([P, SB // P, P], f32r, tag=f"v2h{j}")
                        for tb in range(SB // P):
                            ps = cbt.tile([P, P], f32, tag="cb")
                            nc.tensor.transpose(
                                ps[:], v2fm[:, tb * P : (tb + 1) * P], ident[:]
                            )
                            if tb % 2 == 0:
                                nc.scalar.copy(v2h[:, tb, :], ps[:])
                            else:
                                nc.vector.tensor_copy(v2h[:, tb, :], ps[:])
                        k2hs.append(k2h)
                        v2hs.append(v2h)

                    # batch-major: finish all of batch b, ship its AllGather
                    # half while batch 1-b computes
                    for b in range(B):
                        for j in range(NH):
                            k2h, v2h = k2hs[j], v2hs[j]
                            q2 = zH[:, j, b * S : (b + 1) * S]
                            for sbi in range(S // SBLK):
                                s0 = sbi * SBLK
                                ntt = (s0 + SBLK) // P
                                pv = pvps.tile([P, SBLK], f32, tag="pv")
                                sm = smps.tile([1, SBLK], f32, tag="sm")
                                for tt in range(ntt):
                                    t0 = tt * P
                                    st = stps.tile([P, SBLK], f32, tag="st")
                                    nc.tensor.matmul(
                                        st[:],
                                        lhsT=r(
                                            k2h[:, b * S + t0 : b * S + t0 + P]
                                        ),
                                        rhs=q2[:, s0 : s0 + SBLK],
                                        start=True,
                                        stop=True,
                                    )
                                    e = ep.tile([P, SBLK], f32r, tag="e")
                                    nc.scalar.activation(e[:], st[:], AF.Exp)
                                    if t0 + P - 1 > s0:
                                        off = 640 + (s0 - t0)
                                        nc.vector.tensor_mul(
                                            e[:], e[:],
                                            stepm_sb[:, off : off + SBLK],
                                        )
                                    nc.tensor.matmul(
                                        pv[:],
                                        lhsT=v2h[:, b * (S // P) + tt, :],
                                        rhs=e[:],
                                        start=(tt == 0),
                                        stop=(tt == ntt - 1),
                                    )
                                    nc.tensor.matmul(
                                        sm[:],
                                        lhsT=ones_col[:],
                                        rhs=e[:],
                                        start=(tt == 0),
                                        stop=(tt == ntt - 1),
                                    )
                                rc2 = smp.tile([1, SBLK], f32, tag="rc2")
                                nc.vector.reciprocal_approx_fast(rc2[:], sm[:])
                                rps = smp.tile([P, SBLK], f32, tag="rps")
                                nc.gpsimd.partition_broadcast(rps[:], rc2[:])
                                tmp = smp.tile([P, SBLK], f32, tag="tmp")
                                nc.vector.tensor_mul(tmp[:], pv[:], rps[:])
                                dst = slice(b * S + s0, b * S + s0 + SBLK)
                                nc.vector.tensor_add(
                                    zH[0:HD, j, dst], zH[0:HD, j, dst],
                                    tmp[0:HD, :],
                                )
                                nc.vector.tensor_add(
                                    zH[HD:P, j, dst], zH[HD:P, j, dst],
                                    tmp[HD:P, :],
                                )
                        # cast batch b to bf16 and ship its AllGather half
                        zcH = agp.tile([P, NH, S], bf16, tag="zcH")
                        for j in range(NH):
                            nc.vector.tensor_copy(
                                zcH[:, j, :], zH[:, j, b * S : (b + 1) * S]
                            )
                            nc.gpsimd.dma_start(agh_in[b][j, :, :], zcH[:, j, :])
                        nc.gpsimd.collective_compute(
                            "AllGather",
                            mybir.AluOpType.bypass,
                            replica_groups=RG,
                            ins=[agh_in[b][:, :, :].opt()],
                            outs=[agh_out[b][:, :, :].opt()],
                        )

            # ======== Phase C: FF (my 128 output dims) + ComplexNorm,
            # emission-interleaved per token half so half-b FF and stats run
            # on tensor/sync while half-a norm runs on vector/scalar/gpsimd,
            # letting half-a readout start as early as possible ========
            with (
                tc.tile_pool(name="ffw", bufs=1) as fwp,
                tc.tile_pool(name="zf", bufs=3) as zfp,
                tc.tile_pool(name="fo", bufs=1) as fop,
                tc.tile_pool(name="z2", bufs=1) as z2p,
                tc.tile_pool(name="w2", bufs=3) as wp,
                tc.tile_pool(name="ob", bufs=3) as op_,
            ):
                faS = fwp.tile([P, KT, P], bf16, tag="fa")
                fbS = fwp.tile([P, KT, P], bf16, tag="fb")
                for kt in range(KT):
                    nc.sync.dma_start(faS[:, kt, :], ffA[kt, :, :])
                    nc.sync.dma_start(fbS[:, kt, :], ffB[kt, :, :])

                frh, fih, sqh, fmh = {}, {}, {}, {}
                ctx_c = {}

                def ff_chains(hf):
                    """FF matmul chains + psum drain + magnitude + partial
                    stats + stats-AllGather trigger for one token half."""
                    fre = fop.tile([P, S], f32, tag=f"fre{hf}")
                    fim = fop.tile([P, S], f32, tag=f"fim{hf}")
                    pre = ctx_c["ffps"].tile([P, 2, SBLK], f32, tag="pre")
                    pim = ctx_c["ffps"].tile([P, 2, SBLK], f32, tag="pim")
                    for kt in range(KT):
                        zfk = zfp.tile([P, S], bf16, tag="zf")
                        nc.sync.dma_start(zfk[:], agh_out[hf][kt, :, :])
                        for tsl in range(2):
                            sl = slice(tsl * SBLK, (tsl + 1) * SBLK)
                            nc.tensor.matmul(
                                pre[:, tsl, :],
                                lhsT=faS[:, kt, :], rhs=zfk[:, sl],
                                start=(kt == 0), stop=(kt == KT - 1),
                            )
                        for tsl in range(2):
                            sl = slice(tsl * SBLK, (tsl + 1) * SBLK)
                            nc.tensor.matmul(
                                pim[:, tsl, :],
                                lhsT=fbS[:, kt, :], rhs=zfk[:, sl],
                                start=(kt == 0), stop=(kt == KT - 1),
                            )
                    for tsl in range(2):
                        sl = slice(tsl * SBLK, (tsl + 1) * SBLK)
                        nc.scalar.copy(fre[:, sl], pre[:, tsl, :])
                        nc.vector.tensor_copy(fim[:, sl], pim[:, tsl, :])
                    sq = fop.tile([P, S], f32r, tag=f"sq{hf}")
                    nc.vector.tensor_mul(sq[:], fre[:], fre[:])
                    sq2 = fop.tile([P, S], f32, tag="sq2")
                    nc.gpsimd.tensor_mul(sq2[:], fim[:], fim[:])
                    nc.vector.tensor_add(sq[:], sq[:], sq2[:])
                    fm = fop.tile([P, S], f32r, tag=f"fm{hf}")
                    nc.scalar.activation(fm[:], sq[:], AF.Sqrt)
                    st0 = fop.tile([1, S], f32, tag=f"st0{hf}")
                    st1 = fop.tile([1, S], f32, tag=f"st1{hf}")
                    for tg in range(S // SBLK):
                        sl = slice(tg * SBLK, (tg + 1) * SBLK)
                        psum_c = ctx_c["sps"].tile([1, SBLK], f32, tag="pst")
                        nc.tensor.matmul(
                            psum_c[:], lhsT=ones_col[:], rhs=fm[:, sl],
                            start=True, stop=True,
                        )
                        nc.scalar.copy(st0[:, sl], psum_c[:])
                        psum_c = ctx_c["sps"].tile([1, SBLK], f32, tag="pst")
                        nc.tensor.matmul(
                            psum_c[:], lhsT=ones_col[:], rhs=sq[:, sl],
                            start=True, stop=True,
                        )
                        nc.scalar.copy(st1[:, sl], psum_c[:])
                    nc.scalar.dma_start(sth_in[hf][0:1, :], st0[:, :])
                    nc.scalar.dma_start(sth_in[hf][1:2, :], st1[:, :])
                    nc.gpsimd.collective_compute(
                        "AllGather",
                        mybir.AluOpType.bypass,
                        replica_groups=RG,
                        ins=[sth_in[hf][:, :].opt()],
                        outs=[sth_out[hf][:, :].opt()],
                    )
                    frh[hf], fih[hf], sqh[hf], fmh[hf] = fre, fim, sq, fm

                def norm_half(hf):
                    """ComplexNorm for one half: reduce the gathered per-rank
                    stats locally, then scale the exact f32 FF outputs; ship
                    the normalized state via AllGather."""
                    fre, fim, fm = frh[hf], fih[hf], fmh[hf]
                    deng = nc.gpsimd if hf else nc.sync
                    statg = z2p.tile([KT, S], f32r, tag="statg")
                    deng.dma_start(statg[:], r(sth_out[hf][:, :]))
                    ssum_t = fop.tile([1, S], f32, tag="ssum_t")
                    ssq_t = fop.tile([1, S], f32, tag="ssq_t")
                    for tg in range(S // SBLK):
                        sl = slice(tg * SBLK, (tg + 1) * SBLK)
                        pc = ctx_c["sps"].tile([1, SBLK], f32, tag="tst")
                        nc.tensor.matmul(
                            pc[:], lhsT=stmask[:, 0:1], rhs=statg[:, sl],
                            start=True, stop=True,
                        )
                        nc.scalar.copy(ssum_t[:, sl], pc[:])
                        pc = ctx_c["sps"].tile([1, SBLK], f32, tag="tst")
                        nc.tensor.matmul(
                            pc[:], lhsT=stmask[:, 1:2], rhs=statg[:, sl],
                            start=True, stop=True,
                        )
                        nc.scalar.copy(ssq_t[:, sl], pc[:])
                    mean = fop.tile([1, S], f32, tag="mean")
                    nc.vector.tensor_scalar_mul(mean[:], ssum_t[:], 1.0 / D)
                    w1 = fop.tile([1, S], f32, tag="w1")
                    nc.vector.tensor_mul(w1[:], mean[:], ssum_t[:])
                    nc.vector.tensor_sub(ssq_t[:], ssq_t[:], w1[:])
                    nc.vector.tensor_scalar_mul(
                        ssq_t[:], ssq_t[:], 1.0 / (D - 1)
                    )
                    stdr = fop.tile([1, S], f32, tag="stdr")
                    nc.scalar.activation(stdr[:], ssq_t[:], AF.Sqrt)
                    nc.vector.tensor_scalar_add(stdr[:], stdr[:], EPS)
                    mrep = fop.tile([P, S], f32, tag="mrep")
                    nc.gpsimd.partition_broadcast(mrep[:], mean[:])
                    srep = fop.tile([P, S], f32, tag="srep")
                    nc.gpsimd.partition_broadcast(srep[:], stdr[:])
                    rrep = fop.tile([P, S], f32, tag="rrep")
                    nc.vector.reciprocal_approx_fast(rrep[:], srep[:])
                    xm = fop.tile([P, S], f32, tag="xm")
                    nc.vector.tensor_sub(xm[:], fm[:], mrep[:])
                    nc.vector.tensor_mul(xm[:], xm[:], rrep[:])
                    nc.scalar.activation(xm[:], xm[:], AF.Tanh)
                    rmt = fop.tile([P, S], f32, tag="rmt")
                    nc.vector.tensor_scalar_add(rmt[:], fm[:], EPS)
                    rmt2 = fop.tile([P, S], f32, tag="rmt2")
                    nc.vector.reciprocal_approx_fast(rmt2[:], rmt[:])
                    nc.vector.tensor_mul(xm[:], xm[:], rmt2[:])
                    zc = fop.tile([P, NH, S], bf16, tag="zc")
                    nc.vector.tensor_mul(zc[:, 0, :], fre[:], xm[:])
                    nc.gpsimd.tensor_mul(zc[:, 1, :], fim[:], xm[:])
                    for j in range(NH):
                        nc.scalar.dma_start(ag2h_in[hf][j, :, :], zc[:, j, :])
                    nc.gpsimd.collective_compute(
                        "AllGather",
                        mybir.AluOpType.bypass,
                        replica_groups=RG,
                        ins=[ag2h_in[hf][:, :, :].opt()],
                        outs=[ag2h_out[hf][:, :, :].opt()],
                    )

                def readout_half(hf):
                    deng = nc.gpsimd if hf else nc.sync
                    z2 = z2p.tile([P, KT, S], bf16, tag=f"z2{hf}")
                    for kb in range(KT):
                        deng.dma_start(z2[:, kb, :], ag2h_out[hf][kb, :, :])
                    for vt in range(NVT):
                        wv = wp.tile([P, KT * P], bf16, tag="w")
                        nc.sync.dma_start(wv[:], w2t[vt, :, :])
                        ps = ctx_c["rops"].tile([P, 2, SBLK], f32, tag="ro")
                        for kb in range(KT):
                            for tg in range(2):
                                nc.tensor.matmul(
                                    ps[:, tg, :],
                                    lhsT=wv[:, kb * P : (kb + 1) * P],
                                    rhs=z2[:, kb, tg * SBLK : (tg + 1) * SBLK],
                                    start=(kb == 0),
                                    stop=(kb == KT - 1),
                                )
                        ob = op_.tile([P, 2, SBLK], f32, tag="ob")
                        nc.vector.tensor_scalar_add(
                            ob[:, :, :], ps[:, :, :], bias_sb[:, vt : vt + 1]
                        )
                        nc.gpsimd.dma_start(
                            outv[vt, :, hf * S : (hf + 1) * S], ob[:, :, :]
                        )

                with tc.tile_pool(name="sps", bufs=2, space="PSUM") as sps:
                    ctx_c["sps"] = sps
                    with tc.tile_pool(
                        name="ffps", bufs=1, space="PSUM"
                    ) as ffps:
                        ctx_c["ffps"] = ffps
                        ff_chains(0)   # half-a FF + stats AG
                        ff_chains(1)   # half-b keeps tensor/sync busy
                        norm_half(0)   # half-a norm + state AG
                        norm_half(1)
                    with tc.tile_pool(
                        name="rops", bufs=2, space="PSUM"
                    ) as rops:
                        ctx_c["rops"] = rops
                        readout_half(0)
                        readout_half(1)

    ctx_lp.__exit__(None, None, None)
    nc.compile()
    return nc


def host_prep(x, emb, q_rot, k_rot, v_rot, ff_real, ff_imag, w_r, b_r, w_i, b_i):
    """Host-side sharding + constant table prep. Returns per-core input maps."""
    from ml_dtypes import bfloat16

    x = np.asarray(x)
    emb = np.asarray(emb, np.float32)
    q_rot = np.asarray(q_rot, np.float32)
    k_rot = np.asarray(k_rot, np.float32)
    v_rot = np.asarray(v_rot, np.float32)
    ff_real = np.asarray(ff_real, np.float32)
    ff_imag = np.asarray(ff_imag, np.float32)
    w_r = np.asarray(w_r, np.float32)
    w_i = np.asarray(w_i, np.float32)
    bias = (np.asarray(b_r, np.float32) + np.asarray(b_i, np.float32))

    pos = np.arange(S, dtype=np.float32)[:, None]
    dim = np.arange(D, dtype=np.float32)[None, :]
    freq = np.exp(-(dim / D) * np.float32(math.log(10000.0)))
    ph = pos * freq * np.float32(math.pi)
    cph = np.tile(np.cos(ph), (B, 1)).astype(np.float32)   # [SB, D]
    sph = np.tile(np.sin(ph), (B, 1)).astype(np.float32)

    delta = q_rot - k_rot
    kc, ks = np.cos(delta), np.sin(delta)
    vcos, vsin = np.cos(v_rot), np.sin(v_rot)
    mk = np.zeros((H, 2 * HD, 2 * HD), np.float32)
    mv = np.zeros((H, 2 * HD, 2 * HD), np.float32)
    ar = np.arange(HD)
    for h in range(H):
        mk[h][ar, ar] = kc[h]
        mk[h][HD + ar, ar] = ks[h]
        mk[h][HD + ar, HD + ar] = kc[h]
        mk[h][ar, HD + ar] = -ks[h]
        mv[h][ar, ar] = vcos[h]
        mv[h][HD + ar, ar] = -vsin[h]
        mv[h][ar, HD + ar] = vsin[h]
        mv[h][HD + ar, HD + ar] = vcos[h]

    stmask = np.zeros((KT, 2), np.float32)
    stmask[0::2, 0] = 1.0
    stmask[1::2, 1] = 1.0

    stepm = np.zeros((P, 1280), np.float32)
    ii = np.arange(1280)[None, :] - 640
    stepm[np.arange(P)[:, None] <= ii] = 1.0
    ones = np.ones((P, P), np.float32)

    # ffA[h] = [ff_real rows h*64..; -ff_imag rows], ffB[h] = [ff_imag; ff_real]
    ffA = np.stack(
        [
            np.concatenate(
                [ff_real[h * HD : (h + 1) * HD, :], -ff_imag[h * HD : (h + 1) * HD, :]],
                axis=0,
            )
            for h in range(H)
        ]
    ).astype(np.float32)  # [16, 128, D]
    ffB = np.stack(
        [
            np.concatenate(
                [ff_imag[h * HD : (h + 1) * HD, :], ff_real[h * HD : (h + 1) * HD, :]],
                axis=0,
            )
            for h in range(H)
        ]
    ).astype(np.float32)

    mags = emb[x.reshape(-1)]  # [SB, D] host gather (index routing only)

    per_core = []
    for c in range(NCORES):
        cs = slice(P * c, P * (c + 1))
        # vocab slice, padded to 4096
        wr = np.zeros((D, VCP), np.float32)
        wr[:, :Vc] = w_r[:, Vc * c : Vc * (c + 1)]
        wi = np.zeros((D, VCP), np.float32)
        wi[:, :Vc] = w_i[:, Vc * c : Vc * (c + 1)]
        wstack = np.empty((KT, P, VCP), np.float32)
        for rr in range(NCORES):
            wstack[2 * rr] = wr[P * rr : P * (rr + 1), :]
            wstack[2 * rr + 1] = wi[P * rr : P * (rr + 1), :]
        # [vt, p, kb*128+m]
        w2 = np.ascontiguousarray(
            wstack.reshape(KT, P, NVT, P).transpose(2, 1, 0, 3).reshape(NVT, P, KT * P)
        ).astype(bfloat16)
        bb = np.zeros((VCP,), np.float32)
        bb[:Vc] = bias[Vc * c : Vc * (c + 1)]
        bias2 = np.ascontiguousarray(bb.reshape(NVT, P).T)

        per_core.append(
            dict(
                mg=np.ascontiguousarray(mags[:, cs]),
                cphc=np.ascontiguousarray(cph[:, cs]),
                sphc=np.ascontiguousarray(sph[:, cs]),
                mk=np.ascontiguousarray(mk[2 * c : 2 * c + 2]),
                mv=np.ascontiguousarray(mv[2 * c : 2 * c + 2]),
                stepm=stepm,
                ones=ones,
                ffA=np.ascontiguousarray(ffA[:, :, cs]).astype(bfloat16),
                ffB=np.ascontiguousarray(ffB[:, :, cs]).astype(bfloat16),
                w2t=w2,
                bias2=bias2,
                stmask=stmask,
            )
        )
    return per_core


_NC_CACHE = {}


def get_nc():
    if "nc" not in _NC_CACHE:
        _NC_CACHE["nc"] = build_nc()
    return _NC_CACHE["nc"]


def kernel(x, emb, q_rot, k_rot, v_rot, ff_real, ff_imag, w_r, b_r, w_i, b_i):
    from concourse.bass_utils import run_bass_kernel_spmd

    in_maps = host_prep(
        x, emb, q_rot, k_rot, v_rot, ff_real, ff_imag, w_r, b_r, w_i, b_i
    )
    nc = get_nc()
    res = run_bass_kernel_spmd(nc, in_maps, core_ids=list(range(NCORES)))
    # outv per core: [NVT, P, SB] vocab-major -> [SB, Vc] token-major slice
    chunks = [
        res.results[c]["outv"].reshape(VCP, SB)[:Vc, :].T for c in range(NCORES)
    ]
    logits = np.concatenate(chunks, axis=1).reshape(B, S, V)
    return np.ascontiguousarray(logits.astype(np.float32))

